# revision 1
# baseline (speedup 1.0000x reference)
"""MultiHeadAttention (relu pre-act, softmax, output proj + relu) on 8
Trainium2 NeuronCores via Bass/Tile.

Sharding: each core owns 512 query rows (S/4) of one batch (B=2 -> 4 cores
per batch) across ALL 16 heads; k/v of the batch are replicated on its 4
cores. The output projection is then fully local (no cross-device
reduction) -- the host only concatenates the 8 output slices.

Per-core layout (host pre-transposed, bf16; raw values -- relu on chip):
  qT  [H, DH, 512]   kT [H, DH, S]   v [H, S, DH]
  woT [D, D] (= w_o_w.T)             wob [128, 8] fp32 (partition-major)
  out: outT [D, 512] fp32 (host transposes back)

Math per head (S^T layout so softmax reductions ride the matmuls):
  S^T[k,q] = relu(kT).T @ relu(qT)             PE, K=64, PSUM [128,4*512]
  P^T      = exp(S^T / 8)                      ACT (scores >= 0, no max sub)
  pv       = [relu(V) | 1s]^T @ P^T            PE accum over 16 key chunks:
             rows 0:64 = attnT, rows 64:128 = sumexp replicated 64x
  attnT/sumexp -> am tiles                     DVE reciprocal + mult
  outT     = relu(woT.T @ am + b)              PE + DVE (bias per-partition)
"""

import sys

import numpy as np

try:
    import concourse.bass as bass
except ImportError:  # containers ship the repo here
    sys.path.insert(0, "/opt/trn_rl_repo")
    import concourse.bass as bass

import ml_dtypes

import concourse.mybir as mybir
import concourse.tile as tile
from concourse import bacc
from concourse.bass_utils import run_bass_kernel_spmd

B, S, D, H, DH = 2, 2048, 1024, 16, 64
# exp(s/8) = 2^(s*0.18034): bf16 Schraudolph constants for the DVE path
SCHRAU_A = 0.125 * 1.4426950408889634 * 128.0
SCHRAU_B = 16256.0 - 5.5
_SCHRAU_SETS = {
    0: (),
    3: (3, 8, 13),
    4: (2, 6, 10, 14),
    5: (2, 5, 8, 11, 14),
    6: (1, 4, 6, 9, 12, 14),
    8: (1, 3, 5, 7, 9, 11, 13, 15),
}
import os as _os
QK_FP8 = _os.environ.get("QK_FP8", "1") == "1"
_sk = _os.environ.get("SCHRAU_K", "0")
SCHRAU_ALT = _sk == "alt"
SCHRAU_HEADS = frozenset() if SCHRAU_ALT else frozenset(_SCHRAU_SETS[int(_sk)])
NCORES = 8
SC = S // (NCORES // B)  # 512 query rows per core
NKC = S // 128  # 16 key chunks
BF16 = mybir.dt.bfloat16
FP32 = mybir.dt.float32

LAST_RESULTS = None  # BassKernelResults of the most recent run (for test.py)
_CACHED_NC = None


def _build_nc():
    nc = bacc.Bacc("TRN2", target_bir_lowering=False, debug=False)

    if QK_FP8:
        qT_d = nc.dram_tensor("qT", [H, DH // 2, 2, SC], BF16, kind="ExternalInput").ap()
        kT_d = nc.dram_tensor("kT", [H, DH // 2, 2, S], BF16, kind="ExternalInput").ap()
    else:
        qT_d = nc.dram_tensor("qT", [H, DH, SC], BF16, kind="ExternalInput").ap()
        kT_d = nc.dram_tensor("kT", [H, DH, S], BF16, kind="ExternalInput").ap()
    FP8 = mybir.dt.float8e4
    v_d = nc.dram_tensor("v", [H, 128, S // 128, DH], BF16, kind="ExternalInput").ap()
    woT_d = nc.dram_tensor("woT", [128, 8, D], BF16, kind="ExternalInput").ap()
    wob_d = nc.dram_tensor("wob", [128, 8], FP32, kind="ExternalInput").ap()
    outT_d = nc.dram_tensor("outT", [D, SC], FP32, kind="ExternalOutput").ap()

    AF = mybir.ActivationFunctionType
    _relu_eng = nc.gpsimd if _os.environ.get("RELU_POOL", "0") == "1" else nc.vector
    ALU = mybir.AluOpType

    with tile.TileContext(nc) as tc:
        with (
            tc.tile_pool(name="const", bufs=1) as cpool,
            tc.tile_pool(name="io", bufs=3) as iopool,
            tc.tile_pool(name="pt", bufs=4) as ptpool,
            tc.tile_pool(name="persist", bufs=1) as perpool,
            tc.tile_pool(name="outp", bufs=3) as outpool,
            tc.tile_pool(name="psum", bufs=1, space="PSUM") as pspool,
        ):
            w_sb = cpool.tile([128, 8, D], BF16)  # w_sb[p,c,o] = woT[c*128+p, o]
            nc.sync.dma_start(out=w_sb, in_=woT_d)
            bias_sb = cpool.tile([128, 8], FP32)
            nc.sync.dma_start(out=bias_sb, in_=wob_d)

            # merged attn^T [D_in-part, chunk, query]; head h -> rows
            # 64*(h%2) of chunk h//2. Persists until the projection.
            am_sb = perpool.tile([128, 8, SC], BF16)

            for h in range(H):
                if QK_FP8:
                    # host delivers [32, 2, N] (two dh rows paired per
                    # partition, for the DoubleRow matmul; the pairing only
                    # has to match between lhsT and rhs).
                    kT_raw = iopool.tile([DH // 2, 2, S], BF16, tag="kT_raw")
                    nc.sync.dma_start(out=kT_raw, in_=kT_d[h])
                    qT_raw = iopool.tile([DH // 2, 2, SC], BF16, tag="qT_raw")
                    nc.sync.dma_start(out=qT_raw, in_=qT_d[h])
                    kT_sb = iopool.tile([DH // 2, 2, S], FP8, tag="kT_sb")
                    _relu_eng.tensor_scalar_max(out=kT_sb, in0=kT_raw, scalar1=0.0)
                    qT_sb = iopool.tile([DH // 2, 2, SC], FP8, tag="qT_sb")
                    _relu_eng.tensor_scalar_max(out=qT_sb, in0=qT_raw, scalar1=0.0)
                else:
                    kT_raw = iopool.tile([DH, S], BF16, tag="kT_raw")
                    nc.sync.dma_start(out=kT_raw, in_=kT_d[h])
                    qT_raw = iopool.tile([DH, SC], BF16, tag="qT_raw")
                    nc.sync.dma_start(out=qT_raw, in_=qT_d[h])
                    kT_sb = iopool.tile([DH, S], BF16, tag="kT_sb")
                    _relu_eng.tensor_scalar_max(out=kT_sb, in0=kT_raw, scalar1=0.0)
                    qT_sb = iopool.tile([DH, SC], BF16, tag="qT_sb")
                    _relu_eng.tensor_scalar_max(out=qT_sb, in0=qT_raw, scalar1=0.0)

                v_raw = iopool.tile([128, NKC, DH], BF16, tag="v_raw")
                nc.sync.dma_start(out=v_raw, in_=v_d[h])
                # [relu(V) | ones]: cols 64:128 all 1.0 so the PV matmul also
                # emits sumexp replicated on out partitions 64:128 for free.
                v_ext = iopool.tile([128, NKC, 2 * DH], BF16, tag="v_ext")
                nc.gpsimd.tensor_scalar_max(
                    out=v_ext[:, :, 0:DH], in0=v_raw, scalar1=0.0
                )
                nc.gpsimd.memset(v_ext[:, :, DH : 2 * DH], 1.0)

                pv_ps = pspool.tile([128, SC], FP32, tag="acc", bufs=2)
                kc0 = 0
                for gi, gsz in enumerate((2, 3, 3, 3, 3, 2)):  # key-chunk
                    # groups, double-buffered so PE computes g+1 during exp(g)
                    st_ps = pspool.tile([128, 3, SC], FP32, tag="st", bufs=2)
                    for c in range(gsz):
                        kc = kc0 + c
                        if QK_FP8:
                            nc.tensor.matmul(
                                st_ps[:, c, :],
                                lhsT=kT_sb[:, :, kc * 128 : (kc + 1) * 128],
                                rhs=qT_sb,
                                start=True,
                                stop=True,
                                perf_mode=mybir.MatmulPerfMode.DoubleRow,
                            )
                        else:
                            nc.tensor.matmul(
                                st_ps[:, c, :],
                                lhsT=kT_sb[:, kc * 128 : (kc + 1) * 128],
                                rhs=qT_sb,
                                start=True,
                                stop=True,
                            )
                    # P^T = exp(S^T/sqrt(DH)); scores >= 0 so fp32 exp is
                    # stable without max-subtraction. A subset of heads can
                    # use a bf16 Schraudolph 2^y bit-trick on the DVE
                    # (one tensor_scalar into uint16 + bitcast) to offload
                    # the ACT -- off by default (scheduler models it slower).
                    if (h in SCHRAU_HEADS) or (SCHRAU_ALT and gi % 2 == 1):
                        pt_u16 = ptpool.tile([128, 3, SC], mybir.dt.uint16, tag="pt")
                        nc.vector.tensor_scalar(
                            out=pt_u16[:, 0:gsz, :],
                            in0=st_ps[:, 0:gsz, :],
                            scalar1=SCHRAU_A,
                            scalar2=SCHRAU_B,
                            op0=ALU.mult,
                            op1=ALU.add,
                        )
                        pt_sb = pt_u16.bitcast(BF16)
                    else:
                        pt_sb = ptpool.tile([128, 3, SC], BF16, tag="pt")
                        nc.scalar.activation(
                            pt_sb[:, 0:gsz, :], st_ps[:, 0:gsz, :], AF.Exp, scale=0.125
                        )
                    for c in range(gsz):
                        kc = kc0 + c
                        nc.tensor.matmul(
                            pv_ps,
                            lhsT=v_ext[:, kc, :],
                            rhs=pt_sb[:, c, :],
                            start=(kc == 0),
                            stop=(kc == NKC - 1),
                        )
                    kc0 += gsz

                rd_sb = iopool.tile([DH, SC], FP32, tag="rd")
                nc.vector.reciprocal(rd_sb, pv_ps[DH : 2 * DH, :])
                r0 = 64 * (h % 2)
                nc.vector.tensor_tensor(
                    out=am_sb[r0 : r0 + DH, h // 2, :],
                    in0=pv_ps[0:DH, :],
                    in1=rd_sb,
                    op=ALU.mult,
                )

            for ot in range(8):
                pr_ps = pspool.tile([128, SC], FP32, tag="acc", bufs=2)
                for ic in range(8):
                    nc.tensor.matmul(
                        pr_ps,
                        lhsT=w_sb[:, ic, ot * 128 : (ot + 1) * 128],
                        rhs=am_sb[:, ic, :],
                        start=(ic == 0),
                        stop=(ic == 7),
                    )
                o_sb = outpool.tile([128, SC], FP32, tag="osb")
                # relu(x + bias[o]) in one DVE pass; bias is per-partition.
                nc.vector.tensor_scalar(
                    out=o_sb,
                    in0=pr_ps,
                    scalar1=bias_sb[:, ot : ot + 1],
                    scalar2=0.0,
                    op0=ALU.add,
                    op1=ALU.max,
                )
                nc.sync.dma_start(
                    out=outT_d[ot * 128 : (ot + 1) * 128, :], in_=o_sb
                )

    nc.compile()
    return nc


def kernel(q, k, v, w_o_w, w_o_b):
    global LAST_RESULTS, _CACHED_NC

    q = np.asarray(q, dtype=np.float32)
    k = np.asarray(k, dtype=np.float32)
    v = np.asarray(v, dtype=np.float32)
    w_o_w = np.asarray(w_o_w, dtype=np.float32)
    w_o_b = np.asarray(w_o_b, dtype=np.float32)

    bf = ml_dtypes.bfloat16
    # [B,S,D] -> [B,H,DH,S] (transposed per head) and [B,H,S,DH]
    qT = np.ascontiguousarray(
        q.reshape(B, S, H, DH).transpose(0, 2, 3, 1).astype(bf)
    )
    kT = np.ascontiguousarray(
        k.reshape(B, S, H, DH).transpose(0, 2, 3, 1).astype(bf)
    )
    # [B,H,128,S/128,DH]: partition-major so per-head DMA rows are 2KB runs
    vh = np.ascontiguousarray(
        v.reshape(B, S // 128, 128, H, DH).transpose(0, 3, 2, 1, 4).astype(bf)
    )
    woT = np.ascontiguousarray(
        w_o_w.T.reshape(8, 128, D).transpose(1, 0, 2).astype(bf)
    )
    wob = np.ascontiguousarray(w_o_b.reshape(8, 128).T)  # [128, 8] fp32

    if QK_FP8:
        qT = qT.reshape(B, H, DH // 2, 2, qT.shape[-1])
        kT = kT.reshape(B, H, DH // 2, 2, S)

    if _CACHED_NC is None:
        _CACHED_NC = _build_nc()
    nc = _CACHED_NC

    in_maps = []
    for c in range(NCORES):
        b = c // (NCORES // B)
        s0 = (c % (NCORES // B)) * SC
        in_maps.append(
            {
                "qT": np.ascontiguousarray(
                    qT[b, ..., s0 : s0 + SC]
                ),
                "kT": kT[b],
                "v": vh[b],
                "woT": woT,
                "wob": wob,
            }
        )

    LAST_RESULTS = run_bass_kernel_spmd(nc, in_maps, core_ids=list(range(NCORES)))

    out = np.empty((B, S, D), dtype=np.float32)
    for c in range(NCORES):
        b = c // (NCORES // B)
        s0 = (c % (NCORES // B)) * SC
        out[b, s0 : s0 + SC, :] = LAST_RESULTS.results[c]["outT"].T
    return out

